# revision 38
# baseline (speedup 1.0000x reference)
"""Multi-head attention block (B=4, S=2048, D=1024, H=16, DH=64) on 8 trn2 cores.

Sharding: tensor-parallel over heads (2 groups of 8) x data-parallel over batch (4).
Core c handles batch c//2, heads (c%2)*8 .. +8. Each core computes a partial
output projection (its 8 heads' contribution to cat @ W0); the host sums the
two partials per batch and adds b0.

Per-core kernel (fp16 inputs, f32 accumulation in PSUM):
  xT   [1024, 2048] f16  x transposed (host-prepped), loaded as [128,512] tiles
  wq/wk/wv [1024, 512] f16,  w0 [512, 1024] f16
  qT/kT: [128(e of head-pair), 512(s-block)] f16 tiles (projection + bias)
  v: s-major with a ones column per head: [128(s), 8*65] f16
  scoresT[key, q] = kT.T @ qT per 128-key chunk, two heads row-tiled at
    base partitions 0/64 -> exp on ACT (scale=1/8) -> f16 et
  PV: ctxT+denominator = [v_h | 1].T @ expT accumulated over key chunks (M=65)
  normalize: DVE row-copy -> DMA partition shift -> recip -> gpsimd broadcast
             -> multiply -> catT f16 (head B shifted to partitions 64-127 by DMA)
  out = catT.T @ w0 accumulated over the 4 head-pairs, emitted per q-block

Schedule: pair-major attention; q/k projection chains for pair p+1 are
interleaved into pair p's attention slots so the PE fills the exp-bound
pipeline instead of running a long projection prologue; the out-projection
for q-block qb is emitted right after the last pair finishes qb.
"""

import os
import sys
from collections import deque

for _p in ("/opt/trn_rl_repo",):
    if _p not in sys.path and os.path.isdir(_p):
        sys.path.insert(0, _p)

import numpy as np

import concourse.bass as bass
import concourse.bacc as bacc_mod
import concourse.mybir as mybir
import concourse.tile as tile
import bass_rust
from concourse.vector_clock import ScopedClock

B, S, D, H, DH = 4, 2048, 1024, 16, 64
NCORES = 8
HL = 8            # heads per core
NP = HL // 2      # head pairs per core
E = HL * DH       # 512 local cat width
QB = 512          # q block (columns per attention block)
NQB = S // QB     # 4
KC = 128          # key chunk
NKC = S // KC     # 16
NDC = D // 128    # 8 contraction chunks for projections
F32 = mybir.dt.float32
F16 = mybir.dt.float16
EXPSCALE = 1.0 / np.sqrt(DH)

_MAXW = 1


def _patched_drain_and_barrier(self, tick_clock, wait_clock):
    """Walrus codegen only supports one sync-wait per CTRL instruction; Tile's
    stock exit drain piles every outstanding processor's sem wait onto a single
    drain. Split them across nops (same engine => program order preserved)."""
    probe = self.nc.sync.nop()
    wait_clock.add_sem_waits(probe.ins, ScopedClock({None: tick_clock.global_clock}))
    si = probe.ins.sync_info
    waits = list(si.on_wait) if si is not None and si.on_wait else []
    if len(waits) > _MAXW:
        probe.ins.sync_info = bass_rust.SyncInfo(on_wait=waits[:_MAXW], on_update=[])
        for i in range(_MAXW, len(waits), _MAXW):
            extra = self.nc.sync.nop()
            extra.ins.sync_info = bass_rust.SyncInfo(
                on_wait=waits[i : i + _MAXW], on_update=[]
            )
    self.nc.sync.drain()
    self.nc.all_engine_barrier()
    popped = self.nc._tile_sem_poison_stack.pop()
    assert popped is self._sem_poison
    self.nc.clear_and_free_semaphores(list(self.sems.allocated().values()))
    self.nc.all_engine_barrier()


tile.TileContext._drain_and_barrier = _patched_drain_and_barrier


def build_nc(debug=False):
    nc = bacc_mod.Bacc()
    # xT is tile-blocked on the host: block (k, sb) = x[b].T[k*128:+128,
    # sb*512:+512] stored contiguously at rows (k*NQB+sb)*128 — every DMA
    # is one linear 128 KB read instead of 128 strided 1 KB bursts
    xT = nc.dram_tensor("xT", [NDC * NQB * 128, QB], F16, kind="ExternalInput")
    wq = nc.dram_tensor("wq", [D, E], F16, kind="ExternalInput")
    wk = nc.dram_tensor("wk", [D, E], F16, kind="ExternalInput")
    wv = nc.dram_tensor("wv", [D, E], F16, kind="ExternalInput")
    bqk = nc.dram_tensor("bqk", [128, 2 * NP], F32, kind="ExternalInput")
    bvr = nc.dram_tensor("bvr", [1, E], F32, kind="ExternalInput")
    w0 = nc.dram_tensor("w0", [E, D], F16, kind="ExternalInput")
    out = nc.dram_tensor("out", [S, D], F32, kind="ExternalOutput")

    with tile.TileContext(nc) as tc:
        with (
            tc.tile_pool(name="plong", bufs=1) as plong,
            tc.tile_pool(name="pqkt", bufs=1) as pqkt,
            tc.tile_pool(name="pcat", bufs=1) as pcat,
            tc.tile_pool(name="pv", bufs=1) as pvpool,
            tc.tile_pool(name="pw0", bufs=1) as pw0,
            tc.tile_pool(name="pxt", bufs=32) as pxt,
            tc.tile_pool(name="pw", bufs=24) as pw,
            tc.tile_pool(name="pexp", bufs=5) as pexp,
            tc.tile_pool(name="psm", bufs=2) as psm,
            tc.tile_pool(name="pout", bufs=4) as pout,
            tc.tile_pool(name="pacc", bufs=2, space="PSUM") as pacc,
            tc.tile_pool(name="psS", bufs=1, space="PSUM") as psSpool,
            tc.tile_pool(name="ppv", bufs=1, space="PSUM") as ppvpool,
        ):
            # ---- persistent small tiles ----
            bqkt = plong.tile([128, 2 * NP], F32, tag="bqkt", name="bqkt")
            nc.sync.dma_start(bqkt[:], bqk[:])
            bvrow = plong.tile([1, E], F32, tag="bvrow", name="bvrow")
            nc.sync.dma_start(bvrow[:], bvr[:])
            bvb = plong.tile([128, E], F32, tag="bvb", name="bvb")
            nc.gpsimd.partition_broadcast(bvb[:], bvrow[:])
            # preload the exp table set while DMAs stream in
            dum = psm.tile([128, 2 * NP], F16, tag="dum", name="dum")
            nc.scalar.activation(
                dum[:], bqkt[:], mybir.ActivationFunctionType.Exp, scale=1.0
            )

            # w0 tiles allocated here, loaded after the attention-critical
            # inputs (first consumer is the out-projection at ~300us)
            w0t = [
                pw0.tile([128, D], F16, tag=f"w0_{p}", name=f"w0_{p}")
                for p in range(NP)
            ]

            # catT tiles per (pair, q-block): [128 (2 heads x 64), 512] f16
            catq = [
                [pcat.tile([128, QB], F16, tag=f"cat{p}_{qb}", name=f"cat{p}_{qb}")
                 for qb in range(NQB)]
                for p in range(NP)
            ]

            # v tiles (s-major, ones column per head)
            vaug = [
                pvpool.tile([128, HL * 65], F16, tag=f"v{sc}", name=f"v{sc}")
                for sc in range(NKC)
            ]

            qt = [[None] * NQB for _ in range(NP)]  # [pair][sb] -> [128, 512] f16
            kt = [[None] * NQB for _ in range(NP)]

            # round-robin input DMAs over four engines' dynamic queues so the
            # inbound streaming isn't paced by one ring's descriptor rate
            def dma_in(out_ap, in_ap):
                nc.sync.dma_start(out_ap, in_ap)

            def load_w(dram):
                ts = []
                for k in range(NDC):
                    t = pw.tile([128, E], F16, tag="w", name="w")
                    dma_in(t[:], dram[k * 128 : (k + 1) * 128, :])
                    ts.append(t)
                return ts

            # DMA order: wq/wk (contiguous 128 KB tiles) and x block 0 unblock
            # the lead-in chains; w0 (first used ~300us in) goes last
            wq_t = load_w(wq)
            wk_t = load_w(wk)
            xts = [[None] * NQB for _ in range(NDC)]

            def load_x(sb):
                for k in range(NDC):
                    t = pxt.tile([128, QB], F16, tag="xt", name="xt")
                    base = (k * NQB + sb) * 128
                    dma_in(t[:], xT[base : base + 128, :])
                    xts[k][sb] = t

            load_x(0)
            wv_t = load_w(wv)
            for sb in range(1, NQB):
                load_x(sb)
            for p in range(NP):
                nc.sync.dma_start(w0t[p][:], w0[p * 128 : (p + 1) * 128, :])

            def emit_qk_chain(wtiles, bias_col, dest, kind, p, sb):
                ps = pacc.tile([128, QB], F32, tag="acc", name="acc")
                for k in range(NDC):
                    nc.tensor.matmul(
                        ps[:],
                        wtiles[k][:, p * 128 : (p + 1) * 128],
                        xts[k][sb][:],
                        start=(k == 0),
                        stop=(k == NDC - 1),
                    )
                t = pqkt.tile([128, QB], F16, tag=f"{kind}{p}{sb}", name="qkt")
                nc.vector.tensor_scalar_add(
                    t[:], ps[:], bqkt[:, bias_col + p : bias_col + p + 1]
                )
                dest[p][sb] = t

            def emit_v_chain(sc):
                ps = pacc.tile([128, E], F32, tag="acc", name="acc")
                for k in range(NDC):
                    nc.tensor.matmul(
                        ps[:],
                        xts[k][sc // 4][:, (sc % 4) * 128 : (sc % 4 + 1) * 128],
                        wv_t[k][:],
                        start=(k == 0),
                        stop=(k == NDC - 1),
                    )
                va = vaug[sc]
                nc.gpsimd.memset(
                    va[:].rearrange("p (h c) -> p h c", c=65)[:, :, 64:65], 1.0
                )
                nc.vector.tensor_add(
                    va[:].rearrange("p (h c) -> p h c", c=65)[:, :, 0:64],
                    ps[:].rearrange("p (h c) -> p h c", c=64),
                    bvb[:].rearrange("p (h c) -> p h c", c=64),
                )

            # ---- lead-in: just what block (0,0) needs: q00 + all pair-0 k ----
            emit_qk_chain(wq_t, 0, qt, "q", 0, 0)
            for sb in range(NQB):
                emit_qk_chain(wk_t, NP, kt, "k", 0, sb)

            # filler queue, consumed inside the attention slots. Order is
            # dependency-critical: v(kc) must be emitted before PV uses it
            # (block (0,0) consumes all 16 v chunks, 3 fillers/slot there) and
            # q0x before block (0,x). Pairs 1..3 projections follow, k first.
            def qk_f(wtiles, col, dest, kind, p, sb):
                return lambda: emit_qk_chain(wtiles, col, dest, kind, p, sb)

            def v_f(sc):
                return lambda: emit_v_chain(sc)

            # arrival-ordered: v chunks 0-3 only need x block 0; q0x and later
            # v chunks wait for x blocks 1-3, which stream in behind them
            filler = deque(
                [v_f(0), v_f(1), v_f(2), v_f(3),
                 qk_f(wq_t, 0, qt, "q", 0, 1),
                 v_f(4), v_f(5), v_f(6), v_f(7),
                 qk_f(wq_t, 0, qt, "q", 0, 2),
                 v_f(8), v_f(9), v_f(10), v_f(11),
                 qk_f(wq_t, 0, qt, "q", 0, 3),
                 v_f(12), v_f(13), v_f(14), v_f(15)]
            )
            for p in range(1, NP):
                for sb in range(NQB):
                    filler.append(qk_f(wk_t, NP, kt, "k", p, sb))
                for sb in range(NQB):
                    filler.append(qk_f(wq_t, 0, qt, "q", p, sb))

            def emit_out_proj(qb):
                # contraction split into row halves: the 0:64 rows (head A of
                # each pair) are written by DVE directly, the 64:128 rows
                # arrive via the partition-shift DMA — doing all head-A
                # partials first gives that DMA time to land
                for sc4 in range(4):
                    for db in range(D // QB):
                        ps = pacc.tile([128, QB], F32, tag="acc", name="acc")
                        for p in range(NP):
                            nc.tensor.matmul(
                                ps[:],
                                catq[p][qb][:, sc4 * 128 : (sc4 + 1) * 128],
                                w0t[p][:, db * QB : (db + 1) * QB],
                                start=(p == 0),
                                stop=(p == NP - 1),
                            )
                        ot = pout.tile([128, QB], F32, tag="ot", name="ot")
                        nc.vector.tensor_copy(ot[:], ps[:])
                        sc = qb * 4 + sc4
                        nc.sync.dma_start(
                            out[sc * 128 : (sc + 1) * 128,
                                db * QB : (db + 1) * QB],
                            ot[:],
                        )

            # ---- attention: flat software pipeline over 128 (block, kcg)
            # slots. PV trails the exp by PVLAG slots and flows across block
            # boundaries, so block transitions never bunch PE work in front
            # of the next scores group (which would starve the ACT engine).
            PVLAG = 2
            slot = 0
            pvq = deque()     # (p, qb, kcg, et-pair)
            pvtiles = {}      # live block -> pv psum tiles

            def emit_pv(p, pv, et, kcg):
                for j in range(2):
                    kc = kcg * 2 + j
                    for sub in range(2):
                        h = p * 2 + sub
                        nc.tensor.matmul(
                            pv[sub][:],
                            vaug[kc][:, h * 65 : (h + 1) * 65],
                            et[sub][:, j * QB : (j + 1) * QB],
                            start=(kc == 0),
                            stop=(kc == NKC - 1),
                        )

            def finish_block(p, qb, pv):
                # normalize: row 64 of pv = softmax denominator. The two
                # heads' chains are interleaved stage-by-stage (copies, then
                # both shift DMAs in flight together, recips, broadcasts,
                # muls) so sub1 trails sub0 by one stage instead of a whole
                # serial chain — no PE instructions here, so this cannot
                # perturb the matmul pipeline
                dsb, srow, rrow, rb = [], [], [], []
                for sub in range(2):
                    t = psm.tile([128, QB], F32, tag="dsb", name="dsb")
                    nc.vector.tensor_copy(t[64:65, :], pv[sub][64:65, :])
                    dsb.append(t)
                for sub in range(2):
                    t = psm.tile([1, QB], F32, tag="srow", name="srow")
                    nc.sync.dma_start(t[:], dsb[sub][64:65, :])
                    srow.append(t)
                for sub in range(2):
                    t = psm.tile([1, QB], F32, tag="rrow", name="rrow")
                    nc.vector.reciprocal_approx_fast(t[:], srow[sub][:])
                    rrow.append(t)
                for sub in range(2):
                    t = psm.tile([64, QB], F32, tag="rb", name="rb")
                    nc.gpsimd.partition_broadcast(t[:], rrow[sub][:])
                    rb.append(t)
                nc.vector.tensor_mul(catq[p][qb][0:64, :], pv[0][0:64, :], rb[0][:])
                tb = psm.tile([64, QB], F16, tag="tb", name="tb")
                nc.vector.tensor_mul(tb[:], pv[1][0:64, :], rb[1][:])
                nc.sync.dma_start(catq[p][qb][64:128, :], tb[:])
                if p == NP - 1:
                    emit_out_proj(qb)

            def pop_pv():
                p_, qb_, kcg_, et_ = pvq.popleft()
                b = p_ * NQB + qb_
                if b not in pvtiles:
                    pvtiles[b] = [
                        ppvpool.tile([65, QB], F32, tag=f"pv{sub}",
                                     name=f"pv{sub}")
                        for sub in range(2)
                    ]
                emit_pv(p_, pvtiles[b], et_, kcg_)
                if kcg_ == NKC // 2 - 1:
                    finish_block(p_, qb_, pvtiles.pop(b))

            for p in range(NP):
                for qb in range(NQB):
                    qtile = qt[p][qb]
                    for kcg in range(NKC // 2):
                        psS = [
                            psSpool.tile(
                                [128, 1024], F32, tag=f"psS{sub}", name=f"psS{sub}"
                            )
                            for sub in range(2)
                        ]
                        for j in range(2):
                            kc = kcg * 2 + j
                            ktile = kt[p][kc // 4]
                            ksl = slice((kc % 4) * 128, (kc % 4) * 128 + 128)
                            for sub in range(2):
                                rows = slice(sub * 64, sub * 64 + 64)
                                nc.tensor.matmul(
                                    psS[sub][:, j * QB : (j + 1) * QB],
                                    ktile[rows, ksl],
                                    qtile[rows, :],
                                    start=True,
                                    stop=True,
                                )
                        et = [
                            pexp.tile(
                                [128, 1024], F16, tag=f"e{sub}", name=f"e{sub}"
                            )
                            for sub in range(2)
                        ]
                        for sub in range(2):
                            nc.scalar.activation(
                                et[sub][:],
                                psS[sub][:],
                                mybir.ActivationFunctionType.Exp,
                                scale=EXPSCALE,
                            )
                        # fill the PE while ACT works; block (0,0) also
                        # projects v, which its own PV consumes: 3 per slot
                        npop = 3 if (p == 0 and qb == 0) else (slot % 2 == 0)
                        for _ in range(npop):
                            if filler:
                                filler.popleft()()
                        slot += 1
                        pvq.append((p, qb, kcg, et))
                        # final block: drain eagerly so the trailing PV +
                        # normalize + out-proj overlap its last exp slots
                        last = p == NP - 1 and qb == NQB - 1
                        lag = min(PVLAG, NKC // 2 - 1 - kcg) if last else PVLAG
                        while len(pvq) > lag:
                            pop_pv()
            while pvq:
                pop_pv()
    nc.finalize()
    return nc


_NC_CACHE = None


def _get_nc():
    global _NC_CACHE
    if _NC_CACHE is None:
        _NC_CACHE = build_nc()
    return _NC_CACHE


def make_in_maps(x, Wq, bq, Wk, bk, Wv, bv, W0, b0):
    x = np.asarray(x, dtype=np.float32)
    in_maps = []
    # tile-block xT: block (k, sb) contiguous at rows (k*NQB+sb)*128
    xTb = [
        np.ascontiguousarray(
            x[b].T.astype(np.float16)
            .reshape(NDC, 128, NQB, QB)
            .transpose(0, 2, 1, 3)
            .reshape(NDC * NQB * 128, QB)
        )
        for b in range(B)
    ]
    for c in range(NCORES):
        b = c // 2
        h0 = (c % 2) * HL
        sl = slice(h0, h0 + HL)
        wq_c = np.ascontiguousarray(
            np.asarray(Wq[sl], np.float32).transpose(1, 0, 2).reshape(D, E)
        ).astype(np.float16)
        wk_c = np.ascontiguousarray(
            np.asarray(Wk[sl], np.float32).transpose(1, 0, 2).reshape(D, E)
        ).astype(np.float16)
        wv_c = np.ascontiguousarray(
            np.asarray(Wv[sl], np.float32).transpose(1, 0, 2).reshape(D, E)
        ).astype(np.float16)
        bq_c = np.asarray(bq[sl], np.float32).reshape(E)
        bk_c = np.asarray(bk[sl], np.float32).reshape(E)
        bqk_c = np.empty((128, 2 * NP), np.float32)
        for g in range(NP):
            bqk_c[:, g] = bq_c[g * 128 : (g + 1) * 128]
            bqk_c[:, NP + g] = bk_c[g * 128 : (g + 1) * 128]
        bv_c = np.asarray(bv[sl], np.float32).reshape(1, E)
        w0_c = np.ascontiguousarray(
            np.asarray(W0[h0 * DH : (h0 + HL) * DH], np.float32).astype(np.float16)
        )
        in_maps.append(
            {
                "xT": xTb[b],
                "wq": wq_c,
                "wk": wk_c,
                "wv": wv_c,
                "bqk": bqk_c,
                "bvr": bv_c,
                "w0": w0_c,
            }
        )
    return in_maps


def combine(results, b0):
    out = np.empty((B, S, D), np.float32)
    for b in range(B):
        out[b] = results[2 * b]["out"] + results[2 * b + 1]["out"]
    out += np.asarray(b0, np.float32)[None, None, :]
    return out


def kernel(x, Wq, bq, Wk, bk, Wv, bv, W0, b0):
    from concourse.bass_utils import run_bass_kernel_spmd

    nc = _get_nc()
    in_maps = make_in_maps(x, Wq, bq, Wk, bk, Wv, bv, W0, b0)
    res = run_bass_kernel_spmd(nc, in_maps, core_ids=list(range(NCORES)))
    return combine(res.results, b0)
